# revision 1
# baseline (speedup 1.0000x reference)
"""Trainium2 Bass kernel for nn_AttentionModule (Bahdanau-style attention).

Reference computation (S=512, B=64, H=1024, F=2H):
    cat    = concat([hidden bcast to (S,B,H), encoder_states], -1)      [S,B,2H]
    scores = tanh(cat @ W_attn.T + b_attn) @ W_attn2.T + b_attn2        [S,B,1]
    attn   = softmax(scores[..., 0].T, axis=-1)                         [B,S]
    applied= einsum("bs,sbh->bh", attn, encoder_states)                 [B,H]
    out    = tanh(concat([decoder_out, applied], -1) @ W_comb.T + b_comb)

Sharding: data-parallel over B across 8 cores (8 batch rows per core).
All heavy math stays on-device; the host only slices, transposes and casts
the per-core shards.

Per-core structure:
  - enc_t [8, 1024, 512] bf16: encoder slice with H on partitions. One 2MB
    DMA per batch row (3D access pattern).
  - Main matmul per (b, ft): T^T[f, s] = sum_h W2T[h, f] * encT[h, s] with
    the weight chunk stationary, bf16 at full PE rate, fp32 PSUM.
  - tanh fused on ScalarE with per-partition bias b_attn[f] + hid_part[b, f]
    (hid_part computed on device in a preamble).
  - scores via PE matmul contracting f: lhsT = W_attn2 replicated to 8 cols
    (all psum rows identical -> row b used directly, no partition shifts).
  - softmax over s on 8 partitions (reduce_max(negate) -> Exp with bias and
    fused accumulate -> reciprocal -> scale).
  - attention row broadcast across partitions via a DRAM bounce DMA.
  - applied^T[h, b] on VectorE: multiply resident encT tiles by the broadcast
    attention row, reduce along s. Written column-wise into appT (fp32 output)
    and converted per-b to bf16 for the final matmul.
  - Final combine matmul (bf16) with biases folded as K=1 matmul terms.

Known pitfalls baked into this implementation:
  - bf16 input arrays with tiny rows (16B) get corrupted on the host->device
    path, so every small tensor ships as fp32 and is cast on device.
  - fp32 matmuls run at 1/4 rate; fp32r is full rate but only PE/DMA may
    touch f32r-typed tensors; bf16 everywhere is simplest at full rate.
  - 16/32-bit matmul operand mixing is rejected by the compiler.
  - Multi-dim rearrange DMAs with 16-byte inner blocks corrupt data on HW;
    only used with >=1KB inner blocks here (the encoder load).
"""

import numpy as np

S, B, H = 512, 64, 1024
F = 2 * H
NCORES = 8
BL = B // NCORES          # 8 batch rows per core
KH = H // 128             # 8 contraction chunks over H
KF = F // 128             # 16 feature tiles

_CACHE = {}


def _build(num_devices=NCORES):
    from contextlib import ExitStack

    import concourse.tile as tile
    from concourse import bacc, mybir
    from concourse.masks import make_identity

    f32 = mybir.dt.float32
    bf16 = mybir.dt.bfloat16
    AF = mybir.ActivationFunctionType
    ALU = mybir.AluOpType
    AX = mybir.AxisListType

    nc = bacc.Bacc("TRN2", target_bir_lowering=False, debug=False,
                   num_devices=num_devices)

    enc_t = nc.dram_tensor("enc_t", [BL, H, S], bf16, kind="ExternalInput").ap()
    wat = nc.dram_tensor("wat", [F, F], bf16, kind="ExternalInput").ap()
    wct = nc.dram_tensor("wct", [F, H], bf16, kind="ExternalInput").ap()
    hidT = nc.dram_tensor("hidT", [H, BL], f32, kind="ExternalInput").ap()
    decT = nc.dram_tensor("decT", [H, BL], f32, kind="ExternalInput").ap()
    w2rep = nc.dram_tensor("w2rep", [F, BL], f32, kind="ExternalInput").ap()
    b_attn_d = nc.dram_tensor("b_attn", [1, F], f32, kind="ExternalInput").ap()
    b_comb_d = nc.dram_tensor("b_comb", [1, H], f32, kind="ExternalInput").ap()
    out_d = nc.dram_tensor("out", [BL, H], f32, kind="ExternalOutput").ap()
    appT_d = nc.dram_tensor("appliedT", [H, BL], f32,
                            kind="ExternalOutput").ap()

    with tile.TileContext(nc) as tc:
        with ExitStack() as ctx:
            consts = ctx.enter_context(tc.tile_pool(name="consts", bufs=1))
            enct_pool = ctx.enter_context(tc.tile_pool(name="enct", bufs=2))
            w1_pool = ctx.enter_context(tc.tile_pool(name="w1t", bufs=2))
            tanh_pool = ctx.enter_context(tc.tile_pool(name="tanh", bufs=18))
            attn_pool = ctx.enter_context(tc.tile_pool(name="attn", bufs=2))
            abc_pool = ctx.enter_context(tc.tile_pool(name="abc", bufs=2))
            dram_pool = ctx.enter_context(
                tc.tile_pool(name="dram", bufs=2, space="DRAM"))
            scr_pool = ctx.enter_context(tc.tile_pool(name="scr", bufs=2))
            small_pool = ctx.enter_context(tc.tile_pool(name="small", bufs=4))
            wct_pool = ctx.enter_context(tc.tile_pool(name="wct", bufs=4))
            psT_pool = ctx.enter_context(
                tc.tile_pool(name="psT", bufs=2, space="PSUM"))
            psSc_pool = ctx.enter_context(
                tc.tile_pool(name="psSc", bufs=2, space="PSUM"))
            psPre_pool = ctx.enter_context(
                tc.tile_pool(name="psPre", bufs=2, space="PSUM"))
            psOut_pool = ctx.enter_context(
                tc.tile_pool(name="psOut", bufs=2, space="PSUM"))

            # ---- encoder prefetch for b=0 (emitted first so its DMA leads) --
            def load_enct(b):
                t = enct_pool.tile([128, KH * S], bf16, tag="enct",
                                   name="enct")
                nc.sync.dma_start(
                    t.rearrange("p (k s) -> p k s", s=S),
                    enc_t[b].rearrange("(k p) s -> p k s", p=128))
                return t

            enct_tiles = {0: load_enct(0)}

            # ---- W2T chunk 0 early so the first main matmul can start ----
            w2t_sb = consts.tile([128, KH * F], bf16)
            nc.sync.dma_start(w2t_sb[:, 0:F], wat[H:H + 128, :])

            # ---- small constants (shipped fp32, cast on device) ----
            identity = consts.tile([128, 128], f32)
            make_identity(nc, identity[:])
            ones_bf = consts.tile([1, BL], bf16)
            nc.vector.memset(ones_bf[:], 1.0)
            b_attn_32 = consts.tile([1, F], f32)
            nc.sync.dma_start(b_attn_32[:], b_attn_d[:])
            b_attn_sb = consts.tile([1, F], bf16)
            nc.vector.tensor_copy(b_attn_sb[:], b_attn_32[:])
            b_comb_32 = consts.tile([1, H], f32)
            nc.sync.dma_start(b_comb_32[:], b_comb_d[:])
            b_comb_sb = consts.tile([1, H], bf16)
            nc.vector.tensor_copy(b_comb_sb[:], b_comb_32[:])

            hidT_32 = consts.tile([128, KH * BL], f32)
            decT_32 = consts.tile([128, KH * BL], f32)
            w2rep_32 = consts.tile([128, KF * BL], f32)
            for kc in range(KH):
                nc.sync.dma_start(hidT_32[:, kc * BL:(kc + 1) * BL],
                                  hidT[kc * 128:(kc + 1) * 128, :])
                nc.sync.dma_start(decT_32[:, kc * BL:(kc + 1) * BL],
                                  decT[kc * 128:(kc + 1) * 128, :])
            for ft in range(KF):
                nc.sync.dma_start(w2rep_32[:, ft * BL:(ft + 1) * BL],
                                  w2rep[ft * 128:(ft + 1) * 128, :])
            hidT_sb = consts.tile([128, KH * BL], bf16)
            nc.vector.tensor_copy(hidT_sb[:], hidT_32[:])
            decT_sb = consts.tile([128, KH * BL], bf16)
            nc.vector.tensor_copy(decT_sb[:], decT_32[:])
            w2rep_sb = consts.tile([128, KF * BL], bf16)
            nc.vector.tensor_copy(w2rep_sb[:], w2rep_32[:])

            hidbT_sb = consts.tile([128, KF * BL], f32)
            appT_sb = consts.tile([128, KH * BL], f32)
            appT_bf = consts.tile([128, KH * BL], bf16)

            # ---- hid_part preamble: hidb[b, f] = hidden @ W1.T + b_attn ----
            hidb_row = consts.tile([BL, F], f32)
            for fc in range(F // 512):
                ph = psPre_pool.tile([BL, 512], f32, tag="pre", name=f"ph{fc}")
                for kc in range(KH):
                    w1c = w1_pool.tile([128, 512], bf16, tag="w1t", name="w1c")
                    nc.sync.dma_start(
                        w1c[:], wat[kc * 128:(kc + 1) * 128,
                                    fc * 512:(fc + 1) * 512])
                    nc.tensor.matmul(
                        ph[:], hidT_sb[:, kc * BL:(kc + 1) * BL], w1c[:],
                        start=(kc == 0), stop=False)
                nc.tensor.matmul(
                    ph[:], ones_bf[:], b_attn_sb[:, fc * 512:(fc + 1) * 512],
                    start=False, stop=True)
                nc.vector.tensor_copy(hidb_row[:, fc * 512:(fc + 1) * 512],
                                      ph[:])
            # transpose [8, 2048] -> hidbT_sb [128, KF*8] (f on partitions)
            for ft in range(KF):
                ptp = psPre_pool.tile([128, BL], f32, tag="pre", name="ptp")
                nc.tensor.transpose(ptp[:],
                                    hidb_row[:, ft * 128:(ft + 1) * 128],
                                    identity[:BL, :BL])
                nc.vector.tensor_copy(hidbT_sb[:, ft * BL:(ft + 1) * BL],
                                      ptp[:])

            # ---- remaining W2T chunks ----
            for kc in range(1, KH):
                nc.sync.dma_start(
                    w2t_sb[:, kc * F:(kc + 1) * F],
                    wat[H + kc * 128: H + (kc + 1) * 128, :])

            # ---- main loop over local batch rows ----
            for b in range(BL):
                if b + 1 < BL:
                    enct_tiles[b + 1] = load_enct(b + 1)
                et = enct_tiles.pop(b)

                def etk(kc):
                    return et[:, kc * S:(kc + 1) * S]

                th = []
                for ft in range(KF):
                    pT = psT_pool.tile([128, S], f32, tag="pT", name="pT")
                    for kc in range(KH):
                        nc.tensor.matmul(
                            pT[:],
                            w2t_sb[:, kc * F + ft * 128:
                                   kc * F + (ft + 1) * 128],
                            etk(kc),
                            start=(kc == 0), stop=(kc == KH - 1))
                    t = tanh_pool.tile([128, S], bf16, tag="tanh", name="tanh")
                    nc.scalar.activation(
                        t[:], pT[:], AF.Tanh,
                        bias=hidbT_sb[:, ft * BL + b: ft * BL + b + 1],
                        scale=1.0)
                    th.append(t)

                psc = psSc_pool.tile([BL, S], f32, tag="psc", name="psc")
                for ft in range(KF):
                    nc.tensor.matmul(
                        psc[:],
                        w2rep_sb[:, ft * BL:(ft + 1) * BL],
                        th[ft][:],
                        start=(ft == 0), stop=(ft == KF - 1))

                negmax = small_pool.tile([BL, 1], f32, tag="negmax",
                                         name="negmax")
                nc.vector.reduce_max(negmax[:], psc[:], axis=AX.X, negate=True)
                attn = attn_pool.tile([BL, S], bf16, tag="attn", name="attn")
                sumexp = small_pool.tile([BL, 1], f32, tag="sumexp",
                                         name="sumexp")
                nc.scalar.activation(attn[:], psc[:], AF.Exp,
                                     bias=negmax[:], scale=1.0,
                                     accum_out=sumexp[:])
                recip = small_pool.tile([BL, 1], f32, tag="recip", name="recip")
                nc.vector.reciprocal(recip[:], sumexp[:])
                nc.vector.tensor_scalar_mul(attn[:], attn[:], recip[:])

                # broadcast attn row across 128 partitions via DRAM bounce
                attn_dr = dram_pool.tile([1, S], bf16, tag="attn_dr",
                                         name="attn_dr")
                nc.sync.dma_start(attn_dr[:], attn[0:1, :])
                abc = abc_pool.tile([128, S], bf16, tag="abc", name="abc")
                nc.sync.dma_start(abc[:],
                                  attn_dr[0:1, :].to_broadcast((128, S)))

                for kc in range(KH):
                    scr = scr_pool.tile([128, S], f32, tag="scr", name="scr")
                    nc.vector.tensor_tensor(out=scr[:], in0=etk(kc),
                                            in1=abc[:], op=ALU.mult)
                    nc.vector.reduce_sum(
                        appT_sb[:, kc * BL + b: kc * BL + b + 1],
                        scr[:], axis=AX.X)
                nc.vector.tensor_copy(
                    appT_bf.rearrange("p (k b) -> p k b", b=BL)[:, :, b],
                    appT_sb.rearrange("p (k b) -> p k b", b=BL)[:, :, b])

            # ---- final combine: out = tanh([dec | applied] @ Wc.T + b_comb) --
            pouts = [psOut_pool.tile([BL, 512], f32, tag="pout", name=f"po{i}")
                     for i in range(2)]
            for kc in range(2 * KH):
                if kc < KH:
                    lhs = decT_sb[:, kc * BL:(kc + 1) * BL]
                else:
                    lhs = appT_bf[:, (kc - KH) * BL:(kc - KH + 1) * BL]
                w = wct_pool.tile([128, H], bf16, tag="wct", name="wctt")
                nc.sync.dma_start(w[:], wct[kc * 128:(kc + 1) * 128, :])
                for fc in range(2):
                    nc.tensor.matmul(
                        pouts[fc][:], lhs, w[:, fc * 512:(fc + 1) * 512],
                        start=(kc == 0), stop=False)
            for fc in range(2):
                nc.tensor.matmul(
                    pouts[fc][:], ones_bf[:],
                    b_comb_sb[:, fc * 512:(fc + 1) * 512],
                    start=False, stop=True)

            out_sb = consts.tile([BL, H], f32)
            for fc in range(2):
                nc.scalar.activation(out_sb[:, fc * 512:(fc + 1) * 512],
                                     pouts[fc][:], AF.Tanh)
            nc.sync.dma_start(out_d[:], out_sb[:])
            for kc in range(KH):
                nc.sync.dma_start(appT_d[kc * 128:(kc + 1) * 128, :],
                                  appT_sb[:, kc * BL:(kc + 1) * BL])

    nc.compile()
    return nc


def _get_nc():
    if "nc" not in _CACHE:
        _CACHE["nc"] = _build()
    return _CACHE["nc"]


def make_in_maps(inputs):
    import ml_dtypes
    bf = ml_dtypes.bfloat16

    inp = {k: np.asarray(v, dtype=np.float32) for k, v in inputs.items()}
    hidden = inp["hidden"]
    decoder_out = inp["decoder_out"]
    encoder_states = inp["encoder_states"]
    W_attn = inp["W_attn"]
    b_attn = inp["b_attn"]
    W_attn2 = inp["W_attn2"]
    W_comb = inp["W_comb"]
    b_comb = inp["b_comb"]
    # b_attn2 shifts every score equally -> softmax-invariant, unused.

    wat = np.ascontiguousarray(W_attn.T).astype(bf)
    wct = np.ascontiguousarray(W_comb.T).astype(bf)
    w2rep = np.ascontiguousarray(np.repeat(W_attn2.reshape(F, 1), BL, axis=1))
    b_attn_2d = np.ascontiguousarray(b_attn.reshape(1, F))
    b_comb_2d = np.ascontiguousarray(b_comb.reshape(1, H))

    in_maps = []
    for c in range(NCORES):
        sl = slice(c * BL, (c + 1) * BL)
        in_maps.append({
            "enc_t": np.ascontiguousarray(
                encoder_states[:, sl, :].transpose(1, 2, 0)).astype(bf),
            "wat": wat,
            "wct": wct,
            "hidT": np.ascontiguousarray(hidden[sl].T),
            "decT": np.ascontiguousarray(decoder_out[sl].T),
            "w2rep": w2rep,
            "b_attn": b_attn_2d,
            "b_comb": b_comb_2d,
        })
    return in_maps


def kernel(**inputs):
    from concourse.bass_utils import run_bass_kernel_spmd

    in_maps = make_in_maps(inputs)
    nc = _get_nc()
    res = run_bass_kernel_spmd(nc, in_maps, list(range(NCORES)))
    out = np.concatenate([res.results[c]["out"] for c in range(NCORES)], axis=0)
    applied = np.concatenate(
        [np.ascontiguousarray(res.results[c]["appliedT"].T)
         for c in range(NCORES)], axis=0)
    return out.astype(np.float32), applied.astype(np.float32)



# revision 3
# speedup vs baseline: 1.5636x; 1.5636x over previous
"""Trainium2 Bass kernel for nn_AttentionModule (Bahdanau-style attention).

Reference computation (S=512, B=64, H=1024, F=2H):
    cat    = concat([hidden bcast to (S,B,H), encoder_states], -1)      [S,B,2H]
    scores = tanh(cat @ W_attn.T + b_attn) @ W_attn2.T + b_attn2        [S,B,1]
    attn   = softmax(scores[..., 0].T, axis=-1)                         [B,S]
    applied= einsum("bs,sbh->bh", attn, encoder_states)                 [B,H]
    out    = tanh(concat([decoder_out, applied], -1) @ W_comb.T + b_comb)

Sharding: data-parallel over B across 8 cores (8 batch rows per core).

Key implementation points (v2, fp8 DoubleRow):
  - The dominant matmul T^T[f,s] = sum_h W2T[h,f] * encT[h,s] runs in
    fp8e4m3 with MatmulPerfMode.DoubleRow: each instruction contracts 256
    rows (two 128-chunks stacked on AP dim 1), halving both instruction
    count and PE cycles vs bf16. W2 is pre-scaled by 256 on the host so
    its values sit in fp8's normal range; the tanh activation undoes the
    scale via its fused `scale=1/256`.
  - The hidden part of the scores (hid @ W1.T + b_attn) is precomputed
    once per core in bf16 (it is s-independent) and folded into the tanh
    as a per-partition bias.
  - Scores matmul uses W_attn2 replicated to 128 columns, so the psum
    scores tile [128, S] carries the row broadcast for free — softmax
    runs on it directly and its bf16 output is immediately usable as the
    broadcast attention row for the applied step (no DRAM bounce).
  - applied^T accumulates on VectorE with fused tensor_tensor_reduce
    (mult+add in one pass) over a bf16 encoder copy (kept separate from
    the fp8 copy to protect the `applied` error budget).
  - Final combine matmul in bf16 with biases folded as K=1 matmul terms.

Known pitfalls baked in:
  - bf16 input arrays with tiny rows get corrupted on the host->device
    path, so every small tensor ships as fp32 and is cast on device.
  - fp32 matmuls run at 1/4 rate; 16/32-bit matmul operand mixing is
    rejected; DoubleRow requires both operands fp8e4/e5.
  - TRN float8e4 == ml_dtypes.float8_e4m3 (IEEE-ish, max 240), not
    float8_e4m3fn.
"""

import numpy as np

S, B, H = 512, 64, 1024
F = 2 * H
NCORES = 8
BL = B // NCORES          # 8 batch rows per core
KH = H // 128             # 8 contraction chunks over H
KF = F // 128             # 16 feature tiles
W2SCALE = 256.0           # host pre-scale on W2 so fp8 stays in normal range

_CACHE = {}


def _build(num_devices=NCORES):
    from contextlib import ExitStack

    import concourse.tile as tile
    from concourse import bacc, mybir
    from concourse.masks import make_identity

    f32 = mybir.dt.float32
    bf16 = mybir.dt.bfloat16
    f8 = mybir.dt.float8e4
    AF = mybir.ActivationFunctionType
    ALU = mybir.AluOpType
    AX = mybir.AxisListType
    DR = mybir.MatmulPerfMode.DoubleRow

    nc = bacc.Bacc("TRN2", target_bir_lowering=False, debug=False,
                   num_devices=num_devices)

    # enc8/encb free layout per partition: [b, kc, s]; h = kc*128 + p
    enc8_d = nc.dram_tensor("enc8", [128, BL * KH * S], f8,
                            kind="ExternalInput").ap()
    encb_d = nc.dram_tensor("encb", [128, BL * KH * S], bf16,
                            kind="ExternalInput").ap()
    # w2t8 free layout: [kc, f] (values = 256 * W_attn.T[H + kc*128 + p, f])
    w2t8_d = nc.dram_tensor("w2t8", [128, KH * F], f8,
                            kind="ExternalInput").ap()
    wat1 = nc.dram_tensor("wat1", [H, F], bf16, kind="ExternalInput").ap()
    wct = nc.dram_tensor("wct", [F, H], bf16, kind="ExternalInput").ap()
    hidT = nc.dram_tensor("hidT", [H, BL], f32, kind="ExternalInput").ap()
    decT = nc.dram_tensor("decT", [H, BL], f32, kind="ExternalInput").ap()
    w2rep = nc.dram_tensor("w2rep", [F, 128], f32, kind="ExternalInput").ap()
    b_attn_d = nc.dram_tensor("b_attn", [1, F], f32, kind="ExternalInput").ap()
    b_comb_d = nc.dram_tensor("b_comb", [1, H], f32, kind="ExternalInput").ap()
    out_d = nc.dram_tensor("out", [BL, H], f32, kind="ExternalOutput").ap()
    appT_d = nc.dram_tensor("appliedT", [H, BL], f32,
                            kind="ExternalOutput").ap()

    with tile.TileContext(nc) as tc:
        with ExitStack() as ctx:
            consts = ctx.enter_context(tc.tile_pool(name="consts", bufs=1))
            w1_pool = ctx.enter_context(tc.tile_pool(name="w1t", bufs=4))
            tanh_pool = ctx.enter_context(tc.tile_pool(name="tanh", bufs=18))
            attn_pool = ctx.enter_context(tc.tile_pool(name="attn", bufs=2))
            scr_pool = ctx.enter_context(tc.tile_pool(name="scr", bufs=2))
            small_pool = ctx.enter_context(tc.tile_pool(name="small", bufs=4))
            wct_pool = ctx.enter_context(tc.tile_pool(name="wct", bufs=4))
            psT_pool = ctx.enter_context(
                tc.tile_pool(name="psT", bufs=4, space="PSUM"))
            psSc_pool = ctx.enter_context(
                tc.tile_pool(name="psSc", bufs=2, space="PSUM"))
            psPre_pool = ctx.enter_context(
                tc.tile_pool(name="psPre", bufs=2, space="PSUM"))

            # ---- big resident tensors; b=0 slice of enc8 + W2 lead ----
            enc8_sb = consts.tile([128, BL * KH * S], f8)
            encb_sb = consts.tile([128, BL * KH * S], bf16)
            w2t8_sb = consts.tile([128, KH * F], f8)

            def enc8_load(b):
                nc.sync.dma_start(
                    enc8_sb[:, b * KH * S:(b + 1) * KH * S],
                    enc8_d[:, b * KH * S:(b + 1) * KH * S])

            def encb_load(b):
                nc.sync.dma_start(
                    encb_sb[:, b * KH * S:(b + 1) * KH * S],
                    encb_d[:, b * KH * S:(b + 1) * KH * S])

            enc8_load(0)
            nc.sync.dma_start(w2t8_sb[:], w2t8_d[:])

            # 3D views for DoubleRow slicing
            enc8_v = enc8_sb.rearrange("p (b k s) -> p b k s", b=BL, k=KH)
            encb_v = encb_sb.rearrange("p (b k s) -> p b k s", b=BL, k=KH)
            w2t8_v = w2t8_sb.rearrange("p (k f) -> p k f", k=KH)

            # ---- small constants (shipped fp32, cast on device) ----
            identity = consts.tile([128, 128], f32)
            make_identity(nc, identity[:])
            ones_bf = consts.tile([1, BL], bf16)
            nc.vector.memset(ones_bf[:], 1.0)
            b_attn_32 = consts.tile([1, F], f32)
            nc.sync.dma_start(b_attn_32[:], b_attn_d[:])
            b_attn_sb = consts.tile([1, F], bf16)
            nc.vector.tensor_copy(b_attn_sb[:], b_attn_32[:])
            b_comb_32 = consts.tile([1, H], f32)
            nc.sync.dma_start(b_comb_32[:], b_comb_d[:])
            b_comb_sb = consts.tile([1, H], bf16)
            nc.vector.tensor_copy(b_comb_sb[:], b_comb_32[:])

            hidT_32 = consts.tile([128, KH * BL], f32)
            decT_32 = consts.tile([128, KH * BL], f32)
            w2rep_32 = consts.tile([128, KF * 128], f32)
            for kc in range(KH):
                nc.sync.dma_start(hidT_32[:, kc * BL:(kc + 1) * BL],
                                  hidT[kc * 128:(kc + 1) * 128, :])
                nc.sync.dma_start(decT_32[:, kc * BL:(kc + 1) * BL],
                                  decT[kc * 128:(kc + 1) * 128, :])
            for ft in range(KF):
                nc.sync.dma_start(w2rep_32[:, ft * 128:(ft + 1) * 128],
                                  w2rep[ft * 128:(ft + 1) * 128, :])
            hidT_sb = consts.tile([128, KH * BL], bf16)
            nc.vector.tensor_copy(hidT_sb[:], hidT_32[:])
            decT_sb = consts.tile([128, KH * BL], bf16)
            nc.vector.tensor_copy(decT_sb[:], decT_32[:])
            w2rep_sb = consts.tile([128, KF * 128], bf16)
            nc.vector.tensor_copy(w2rep_sb[:], w2rep_32[:])

            hidbT_sb = consts.tile([128, KF * BL], f32)
            appT_sb = consts.tile([128, KH * BL], f32)
            appT_bf = consts.tile([128, KH * BL], bf16)

            # ---- hid_part preamble: hidb[b, f] = hidden @ W1.T + b_attn ----
            hidb_row = consts.tile([BL, F], f32)
            for fc in range(F // 512):
                ph = psPre_pool.tile([BL, 512], f32, tag="pre", name=f"ph{fc}")
                for kc in range(KH):
                    w1c = w1_pool.tile([128, 512], bf16, tag="w1t", name="w1c")
                    nc.sync.dma_start(
                        w1c[:], wat1[kc * 128:(kc + 1) * 128,
                                     fc * 512:(fc + 1) * 512])
                    nc.tensor.matmul(
                        ph[:], hidT_sb[:, kc * BL:(kc + 1) * BL], w1c[:],
                        start=(kc == 0), stop=False)
                nc.tensor.matmul(
                    ph[:], ones_bf[:], b_attn_sb[:, fc * 512:(fc + 1) * 512],
                    start=False, stop=True)
                nc.vector.tensor_copy(hidb_row[:, fc * 512:(fc + 1) * 512],
                                      ph[:])
            # transpose [8, 2048] -> hidbT_sb [128, KF*8] (f on partitions)
            for ft in range(KF):
                ptp = psPre_pool.tile([128, BL], f32, tag="pre", name="ptp")
                nc.tensor.transpose(ptp[:],
                                    hidb_row[:, ft * 128:(ft + 1) * 128],
                                    identity[:BL, :BL])
                nc.vector.tensor_copy(hidbT_sb[:, ft * BL:(ft + 1) * BL],
                                      ptp[:])

            # ---- remaining enc DMAs (b=1.. fp8, then bf16 copies) ----
            for b in range(1, BL):
                enc8_load(b)
            for b in range(BL):
                encb_load(b)

            # ---- main loop over local batch rows ----
            inv = 1.0 / W2SCALE
            for b in range(BL):
                th = [None] * KF
                psc = psSc_pool.tile([128, S], f32, tag="psc", name="psc")

                def attn2(ft):
                    nc.tensor.matmul(
                        psc[:],
                        w2rep_sb[:, ft * 128:(ft + 1) * 128],
                        th[ft][:],
                        start=(ft == 0), stop=(ft == KF - 1))

                for ft in range(KF):
                    pT = psT_pool.tile([128, S], f32, tag="pT", name="pT")
                    for kc2 in range(KH // 2):
                        nc.tensor.matmul(
                            pT[:],
                            w2t8_v[:, 2 * kc2:2 * kc2 + 2,
                                   ft * 128:(ft + 1) * 128],
                            enc8_v[:, b, 2 * kc2:2 * kc2 + 2, :],
                            start=(kc2 == 0), stop=(kc2 == KH // 2 - 1),
                            perf_mode=DR)
                    t = tanh_pool.tile([128, S], bf16, tag="tanh", name="tanh")
                    nc.scalar.activation(
                        t[:], pT[:], AF.Tanh,
                        bias=hidbT_sb[:, ft * BL + b: ft * BL + b + 1],
                        scale=inv)
                    th[ft] = t
                    if ft >= 2:
                        attn2(ft - 2)
                attn2(KF - 2)
                attn2(KF - 1)

                # softmax over s; psc rows are identical (w2rep replication)
                negmax = small_pool.tile([128, 1], f32, tag="negmax",
                                         name="negmax")
                nc.vector.reduce_max(negmax[:], psc[:], axis=AX.X, negate=True)
                attn = attn_pool.tile([128, S], bf16, tag="attn", name="attn")
                sumexp = small_pool.tile([128, 1], f32, tag="sumexp",
                                         name="sumexp")
                nc.scalar.activation(attn[:], psc[:], AF.Exp,
                                     bias=negmax[:], scale=1.0,
                                     accum_out=sumexp[:])
                recip = small_pool.tile([128, 1], f32, tag="recip",
                                        name="recip")
                nc.vector.reciprocal(recip[:], sumexp[:])
                nc.vector.tensor_scalar_mul(attn[:], attn[:], recip[:])

                # applied^T[h, b]: multiply+reduce on VectorE (bf16 for rate)
                for kc in range(KH):
                    scr = scr_pool.tile([128, S], bf16, tag="scr", name="scr")
                    nc.vector.tensor_tensor(out=scr[:], in0=encb_v[:, b, kc, :],
                                            in1=attn[:], op=ALU.mult)
                    nc.vector.reduce_sum(
                        appT_sb[:, kc * BL + b: kc * BL + b + 1],
                        scr[:], axis=AX.X)
                nc.vector.tensor_copy(
                    appT_bf.rearrange("p (k b) -> p k b", b=BL)[:, :, b],
                    appT_sb.rearrange("p (k b) -> p k b", b=BL)[:, :, b])

            # ---- final combine: out = tanh([dec | applied] @ Wc.T + b_comb) --
            pouts = [psPre_pool.tile([BL, 512], f32, tag="pre", name=f"po{i}")
                     for i in range(2)]
            for kc in range(2 * KH):
                if kc < KH:
                    lhs = decT_sb[:, kc * BL:(kc + 1) * BL]
                else:
                    lhs = appT_bf[:, (kc - KH) * BL:(kc - KH + 1) * BL]
                w = wct_pool.tile([128, H], bf16, tag="wct", name="wctt")
                nc.sync.dma_start(w[:], wct[kc * 128:(kc + 1) * 128, :])
                for fc in range(2):
                    nc.tensor.matmul(
                        pouts[fc][:], lhs, w[:, fc * 512:(fc + 1) * 512],
                        start=(kc == 0), stop=False)
            for fc in range(2):
                nc.tensor.matmul(
                    pouts[fc][:], ones_bf[:],
                    b_comb_sb[:, fc * 512:(fc + 1) * 512],
                    start=False, stop=True)

            out_sb = consts.tile([BL, H], f32)
            for fc in range(2):
                nc.scalar.activation(out_sb[:, fc * 512:(fc + 1) * 512],
                                     pouts[fc][:], AF.Tanh)
            nc.sync.dma_start(out_d[:], out_sb[:])
            for kc in range(KH):
                nc.sync.dma_start(appT_d[kc * 128:(kc + 1) * 128, :],
                                  appT_sb[:, kc * BL:(kc + 1) * BL])

    nc.compile()
    return nc


def _get_nc():
    if "nc" not in _CACHE:
        _CACHE["nc"] = _build()
    return _CACHE["nc"]


def make_in_maps(inputs):
    import ml_dtypes
    bf = ml_dtypes.bfloat16
    f8 = ml_dtypes.float8_e4m3

    inp = {k: np.asarray(v, dtype=np.float32) for k, v in inputs.items()}
    hidden = inp["hidden"]
    decoder_out = inp["decoder_out"]
    encoder_states = inp["encoder_states"]
    W_attn = inp["W_attn"]
    b_attn = inp["b_attn"]
    W_attn2 = inp["W_attn2"]
    W_comb = inp["W_comb"]
    b_comb = inp["b_comb"]
    # b_attn2 shifts every score equally -> softmax-invariant, unused.

    wat = np.ascontiguousarray(W_attn.T)                    # [F, F] fp32
    wat1 = np.ascontiguousarray(wat[:H]).astype(bf)         # [H, F]
    # W2 part, scaled, h-chunked to [128, KH*F] fp8
    w2t8 = np.ascontiguousarray(
        (wat[H:] * W2SCALE).reshape(KH, 128, F).transpose(1, 0, 2)
        .reshape(128, KH * F)).astype(f8)
    wct = np.ascontiguousarray(W_comb.T).astype(bf)
    w2rep = np.ascontiguousarray(np.repeat(W_attn2.reshape(F, 1), 128, axis=1))
    b_attn_2d = np.ascontiguousarray(b_attn.reshape(1, F))
    b_comb_2d = np.ascontiguousarray(b_comb.reshape(1, H))

    in_maps = []
    for c in range(NCORES):
        sl = slice(c * BL, (c + 1) * BL)
        # [S, BL, H] -> [BL, H, S] -> [BL, KH, 128, S] -> [128, BL, KH, S]
        enc = np.ascontiguousarray(
            encoder_states[:, sl, :].transpose(1, 2, 0)
            .reshape(BL, KH, 128, S).transpose(2, 0, 1, 3)
            .reshape(128, BL * KH * S))
        in_maps.append({
            "enc8": enc.astype(f8),
            "encb": enc.astype(bf),
            "w2t8": w2t8,
            "wat1": wat1,
            "wct": wct,
            "hidT": np.ascontiguousarray(hidden[sl].T),
            "decT": np.ascontiguousarray(decoder_out[sl].T),
            "w2rep": w2rep,
            "b_attn": b_attn_2d,
            "b_comb": b_comb_2d,
        })
    return in_maps


def kernel(**inputs):
    from concourse.bass_utils import run_bass_kernel_spmd

    in_maps = make_in_maps(inputs)
    nc = _get_nc()
    res = run_bass_kernel_spmd(nc, in_maps, list(range(NCORES)))
    out = np.concatenate([res.results[c]["out"] for c in range(NCORES)], axis=0)
    applied = np.concatenate(
        [np.ascontiguousarray(res.results[c]["appliedT"].T)
         for c in range(NCORES)], axis=0)
    return out.astype(np.float32), applied.astype(np.float32)


# revision 5
# speedup vs baseline: 1.5747x; 1.0071x over previous
"""Trainium2 Bass kernel for nn_AttentionModule (Bahdanau-style attention).

Reference computation (S=512, B=64, H=1024, F=2H):
    cat    = concat([hidden bcast to (S,B,H), encoder_states], -1)      [S,B,2H]
    scores = tanh(cat @ W_attn.T + b_attn) @ W_attn2.T + b_attn2        [S,B,1]
    attn   = softmax(scores[..., 0].T, axis=-1)                         [B,S]
    applied= einsum("bs,sbh->bh", attn, encoder_states)                 [B,H]
    out    = tanh(concat([decoder_out, applied], -1) @ W_comb.T + b_comb)

Sharding: data-parallel over B across 8 cores (8 batch rows per core).

v3 structure (fp8 DoubleRow everywhere it pays):
  - Main matmul T^T[f,s] = sum_h W2T[h,f]*encT[h,s] in fp8e4m3 DoubleRow
    (256 contraction rows per instruction).  W2 host-scaled by 256; the
    tanh undoes it via its fused scale.  W2T is laid out per-f-tile so
    the first DR group only waits on a 128KB DMA.
  - hid@W1.T preamble: W1 loaded once as a resident [128, KH*F] tile (8
    big DMAs, no per-(fc,kc) reloads), 32 bf16 matmuls + PE transposes.
  - tanh outputs land in fp8 pair-tiles [128, 2, S]; the scores matmul
    (attn2) also runs fp8 DoubleRow with W_attn2 replicated to 128
    columns (scaled by 256), so the psum scores tile [128, S] carries
    the row broadcast for free.
  - Softmax skips max-subtraction (scores are provably tiny); Exp fuses
    the 1/256 descale and the sum via accum_out.
  - applied^T: mult+reduce per h-chunk, split 5/3 between VectorE and
    GpSimd, over a bf16 encoder copy (separate from the fp8 copy to
    protect the `applied` error budget).
  - enc fp8/bf16 copies are streamed per batch row (pools, prefetch one
    row ahead) instead of held resident.
  - Final combine bf16; its decoder-half is emitted before the last
    batch row so the PE tail only waits on the last row's applied.
"""

import numpy as np

S, B, H = 512, 64, 1024
F = 2 * H
NCORES = 8
BL = B // NCORES          # 8 batch rows per core
KH = H // 128             # 8 contraction chunks over H
KF = F // 128             # 16 feature tiles
W2SCALE = 256.0           # host pre-scale on W2 / W_attn2 for fp8 range

_CACHE = {}


def _build(num_devices=NCORES):
    from contextlib import ExitStack

    import concourse.tile as tile
    from concourse import bacc, mybir
    from concourse.masks import make_identity

    f32 = mybir.dt.float32
    bf16 = mybir.dt.bfloat16
    f8 = mybir.dt.float8e4
    AF = mybir.ActivationFunctionType
    ALU = mybir.AluOpType
    AX = mybir.AxisListType
    DR = mybir.MatmulPerfMode.DoubleRow

    nc = bacc.Bacc("TRN2", target_bir_lowering=False, debug=False,
                   num_devices=num_devices)

    # enc free layout per partition: [b, kc, s]; h = kc*128 + p
    enc8_d = nc.dram_tensor("enc8", [128, BL * KH * S], f8,
                            kind="ExternalInput").ap()
    encb_d = nc.dram_tensor("encb", [128, BL * KH * S], bf16,
                            kind="ExternalInput").ap()
    # w2t8 free layout: [ft, kc, f]  (values = 256 * W_attn.T[H+kc*128+p,
    # ft*128+f]); wat1 free layout: [kc, f] (W_attn.T[kc*128+p, f])
    w2t8_d = nc.dram_tensor("w2t8", [128, KF * KH * 128], f8,
                            kind="ExternalInput").ap()
    wat1_d = nc.dram_tensor("wat1", [128, KH * F], bf16,
                            kind="ExternalInput").ap()
    # w2rep8 free layout: [ft, c] (values = 256 * W_attn2[ft*128+p], any c)
    w2rep8_d = nc.dram_tensor("w2rep8", [128, KF * 128], f8,
                              kind="ExternalInput").ap()
    wct = nc.dram_tensor("wct", [F, H], bf16, kind="ExternalInput").ap()
    hidT = nc.dram_tensor("hidT", [H, BL], f32, kind="ExternalInput").ap()
    decT = nc.dram_tensor("decT", [H, BL], f32, kind="ExternalInput").ap()
    b_attn_d = nc.dram_tensor("b_attn", [1, F], f32, kind="ExternalInput").ap()
    b_comb_d = nc.dram_tensor("b_comb", [1, H], f32, kind="ExternalInput").ap()
    out_d = nc.dram_tensor("out", [BL, H], f32, kind="ExternalOutput").ap()
    appT_d = nc.dram_tensor("appliedT", [H, BL], f32,
                            kind="ExternalOutput").ap()

    with tile.TileContext(nc) as tc:
        with ExitStack() as ctx:
            consts = ctx.enter_context(tc.tile_pool(name="consts", bufs=1))
            enc8_pool = ctx.enter_context(tc.tile_pool(name="enc8p", bufs=2))
            encb_pool = ctx.enter_context(tc.tile_pool(name="encbp", bufs=2))
            thp_pool = ctx.enter_context(tc.tile_pool(name="thp", bufs=10))
            attn_pool = ctx.enter_context(tc.tile_pool(name="attn", bufs=2))
            scr_pool = ctx.enter_context(tc.tile_pool(name="scr", bufs=2))
            gscr_pool = ctx.enter_context(tc.tile_pool(name="gscr", bufs=2))
            small_pool = ctx.enter_context(tc.tile_pool(name="small", bufs=4))
            wct_pool = ctx.enter_context(tc.tile_pool(name="wct", bufs=4))
            psT_pool = ctx.enter_context(
                tc.tile_pool(name="psT", bufs=4, space="PSUM"))
            psSc_pool = ctx.enter_context(
                tc.tile_pool(name="psSc", bufs=2, space="PSUM"))
            psPre_pool = ctx.enter_context(
                tc.tile_pool(name="psPre", bufs=2, space="PSUM"))

            # ---- leading DMAs: W1 (preamble critical path), W2 head, enc b0
            w1_sb = consts.tile([128, KH * F], bf16)
            for kc in range(KH):
                nc.sync.dma_start(w1_sb[:, kc * F:(kc + 1) * F],
                                  wat1_d[:, kc * F:(kc + 1) * F])
            w2t8_sb = consts.tile([128, KF * KH * 128], f8)
            CW = KH * 128  # per-f-tile chunk width

            def w2_load(ft):
                nc.sync.dma_start(w2t8_sb[:, ft * CW:(ft + 1) * CW],
                                  w2t8_d[:, ft * CW:(ft + 1) * CW])

            w2_load(0)
            w2_load(1)

            def enc8_load(b):
                t = enc8_pool.tile([128, KH * S], f8, tag="enc8", name="enc8")
                half = KH * S // 2
                for i in range(2):
                    nc.sync.dma_start(
                        t[:, i * half:(i + 1) * half],
                        enc8_d[:, b * KH * S + i * half:
                               b * KH * S + (i + 1) * half])
                return t

            def encb_load(b):
                t = encb_pool.tile([128, KH * S], bf16, tag="encb",
                                   name="encb")
                q = KH * S // 4
                for i in range(4):
                    nc.sync.dma_start(
                        t[:, i * q:(i + 1) * q],
                        encb_d[:, b * KH * S + i * q:
                               b * KH * S + (i + 1) * q])
                return t

            enc8_tiles = {0: enc8_load(0)}

            # ---- small constants (shipped fp32, cast on device) ----
            identity = consts.tile([128, 128], f32)
            make_identity(nc, identity[:])
            ones_bf = consts.tile([1, BL], bf16)
            nc.vector.memset(ones_bf[:], 1.0)
            b_attn_32 = consts.tile([1, F], f32)
            nc.sync.dma_start(b_attn_32[:], b_attn_d[:])
            b_attn_sb = consts.tile([1, F], bf16)
            nc.vector.tensor_copy(b_attn_sb[:], b_attn_32[:])
            b_comb_32 = consts.tile([1, H], f32)
            nc.sync.dma_start(b_comb_32[:], b_comb_d[:])
            b_comb_sb = consts.tile([1, H], bf16)
            nc.vector.tensor_copy(b_comb_sb[:], b_comb_32[:])

            hidT_32 = consts.tile([128, KH * BL], f32)
            decT_32 = consts.tile([128, KH * BL], f32)
            for kc in range(KH):
                nc.sync.dma_start(hidT_32[:, kc * BL:(kc + 1) * BL],
                                  hidT[kc * 128:(kc + 1) * 128, :])
                nc.sync.dma_start(decT_32[:, kc * BL:(kc + 1) * BL],
                                  decT[kc * 128:(kc + 1) * 128, :])
            hidT_sb = consts.tile([128, KH * BL], bf16)
            nc.vector.tensor_copy(hidT_sb[:], hidT_32[:])
            decT_sb = consts.tile([128, KH * BL], bf16)
            nc.vector.tensor_copy(decT_sb[:], decT_32[:])
            w2rep8_sb = consts.tile([128, KF * 128], f8)
            nc.sync.dma_start(w2rep8_sb[:], w2rep8_d[:])

            hidbT_sb = consts.tile([128, KF * BL], f32)
            appT_sb = consts.tile([128, KH * BL], f32)
            appT_bf = consts.tile([128, KH * BL], bf16)

            # ---- remaining W2 chunks + enc b0 bf16 ----
            for ft in range(2, KF):
                w2_load(ft)
            encb_tiles = {0: encb_load(0)}

            # ---- hid_part preamble: hidb[b, f] = hidden @ W1.T + b_attn ----
            hidb_row = consts.tile([BL, F], f32)
            for fc in range(F // 512):
                ph = psPre_pool.tile([BL, 512], f32, tag="pre", name=f"ph{fc}")
                for kc in range(KH):
                    nc.tensor.matmul(
                        ph[:], hidT_sb[:, kc * BL:(kc + 1) * BL],
                        w1_sb[:, kc * F + fc * 512: kc * F + (fc + 1) * 512],
                        start=(kc == 0), stop=False)
                nc.tensor.matmul(
                    ph[:], ones_bf[:], b_attn_sb[:, fc * 512:(fc + 1) * 512],
                    start=False, stop=True)
                nc.vector.tensor_copy(hidb_row[:, fc * 512:(fc + 1) * 512],
                                      ph[:])
            # transpose [8, 2048] -> hidbT_sb [128, KF*8] (f on partitions)
            for ft in range(KF):
                ptp = psPre_pool.tile([128, BL], f32, tag="pre", name="ptp")
                nc.tensor.transpose(ptp[:],
                                    hidb_row[:, ft * 128:(ft + 1) * 128],
                                    identity[:BL, :BL])
                nc.vector.tensor_copy(hidbT_sb[:, ft * BL:(ft + 1) * BL],
                                      ptp[:])

            w2t8_v = w2t8_sb.rearrange("p (t k f) -> p t k f", t=KF, k=KH)
            w2rep8_v = w2rep8_sb.rearrange("p (t c) -> p t c", t=KF)

            # ---- main loop over local batch rows ----
            inv = 1.0 / W2SCALE
            pouts = [None, None]
            for b in range(BL):
                if b + 1 < BL:
                    enc8_tiles[b + 1] = enc8_load(b + 1)
                    encb_tiles[b + 1] = encb_load(b + 1)
                et8 = enc8_tiles.pop(b)
                et8_v = et8.rearrange("p (k s) -> p k s", k=KH)

                psc = psSc_pool.tile([128, S], f32, tag="psc", name="psc")
                thp = [None] * (KF // 2)

                def attn2(fp):
                    nc.tensor.matmul(
                        psc[:], w2rep8_v[:, 2 * fp:2 * fp + 2, :],
                        thp[fp][:],
                        start=(fp == 0), stop=(fp == KF // 2 - 1),
                        perf_mode=DR)

                for ft in range(KF):
                    pT = psT_pool.tile([128, S], f32, tag="pT", name="pT")
                    for kc2 in range(KH // 2):
                        nc.tensor.matmul(
                            pT[:],
                            w2t8_v[:, ft, 2 * kc2:2 * kc2 + 2, :],
                            et8_v[:, 2 * kc2:2 * kc2 + 2, :],
                            start=(kc2 == 0), stop=(kc2 == KH // 2 - 1),
                            perf_mode=DR)
                    if ft % 2 == 0:
                        thp[ft // 2] = thp_pool.tile([128, 2, S], f8,
                                                     tag="thp", name="thp")
                    nc.scalar.activation(
                        thp[ft // 2][:, ft % 2, :], pT[:], AF.Tanh,
                        bias=hidbT_sb[:, ft * BL + b: ft * BL + b + 1],
                        scale=inv)
                    # scores matmul, one pair behind the tanh pipeline
                    if ft % 2 == 1 and ft >= 3:
                        attn2(ft // 2 - 1)
                attn2(KF // 2 - 2)
                attn2(KF // 2 - 1)

                # softmax over s (no max subtraction: |scores| <~ 2)
                attn = attn_pool.tile([128, S], bf16, tag="attn", name="attn")
                sumexp = small_pool.tile([128, 1], f32, tag="sumexp",
                                         name="sumexp")
                nc.scalar.activation(attn[:], psc[:], AF.Exp,
                                     bias=0.0, scale=inv,
                                     accum_out=sumexp[:])
                recip = small_pool.tile([128, 1], f32, tag="recip",
                                        name="recip")
                nc.vector.reciprocal(recip[:], sumexp[:])
                nc.vector.tensor_scalar_mul(attn[:], attn[:], recip[:])

                # applied^T[h, b]: mult+reduce, split across VectorE/GpSimd
                etb = encb_tiles.pop(b)
                etb_v = etb.rearrange("p (k s) -> p k s", k=KH)
                # gpsimd can only reduce over partitions, so it handles a
                # share of the multiplies; VectorE does all free-axis reduces
                for kc in range(KH):
                    if kc < 5:
                        eng, pool = nc.vector, scr_pool
                    else:
                        eng, pool = nc.gpsimd, gscr_pool
                    scr = pool.tile([128, S], bf16, tag="scr", name="scr")
                    eng.tensor_tensor(out=scr[:], in0=etb_v[:, kc, :],
                                      in1=attn[:], op=ALU.mult)
                    nc.vector.reduce_sum(
                        appT_sb[:, kc * BL + b: kc * BL + b + 1],
                        scr[:], axis=AX.X)

                # decoder half of the final combine: emit before the last row
                if b == BL - 2:
                    for i in range(2):
                        pouts[i] = psPre_pool.tile([BL, 512], f32, tag="pre",
                                                   name=f"po{i}")
                    for kc in range(KH):
                        w = wct_pool.tile([128, H], bf16, tag="wct",
                                          name="wctt")
                        nc.sync.dma_start(w[:], wct[kc * 128:(kc + 1) * 128, :])
                        for fc in range(2):
                            nc.tensor.matmul(
                                pouts[fc][:],
                                decT_sb[:, kc * BL:(kc + 1) * BL],
                                w[:, fc * 512:(fc + 1) * 512],
                                start=(kc == 0), stop=False)

            # ---- final combine: += applied @ Wc_applied.T, bias, tanh ----
            nc.vector.tensor_copy(appT_bf[:], appT_sb[:])
            for kc in range(KH):
                w = wct_pool.tile([128, H], bf16, tag="wct", name="wctt")
                nc.sync.dma_start(w[:], wct[(KH + kc) * 128:
                                            (KH + kc + 1) * 128, :])
                for fc in range(2):
                    nc.tensor.matmul(
                        pouts[fc][:], appT_bf[:, kc * BL:(kc + 1) * BL],
                        w[:, fc * 512:(fc + 1) * 512],
                        start=False, stop=False)
            for fc in range(2):
                nc.tensor.matmul(
                    pouts[fc][:], ones_bf[:],
                    b_comb_sb[:, fc * 512:(fc + 1) * 512],
                    start=False, stop=True)

            out_sb = consts.tile([BL, H], f32)
            for fc in range(2):
                nc.scalar.activation(out_sb[:, fc * 512:(fc + 1) * 512],
                                     pouts[fc][:], AF.Tanh)
            nc.sync.dma_start(out_d[:], out_sb[:])
            for kc in range(KH):
                nc.sync.dma_start(appT_d[kc * 128:(kc + 1) * 128, :],
                                  appT_sb[:, kc * BL:(kc + 1) * BL])

    nc.compile()
    return nc


def _get_nc():
    if "nc" not in _CACHE:
        _CACHE["nc"] = _build()
    return _CACHE["nc"]


def make_in_maps(inputs):
    import ml_dtypes
    bf = ml_dtypes.bfloat16
    f8 = ml_dtypes.float8_e4m3

    inp = {k: np.asarray(v, dtype=np.float32) for k, v in inputs.items()}
    hidden = inp["hidden"]
    decoder_out = inp["decoder_out"]
    encoder_states = inp["encoder_states"]
    W_attn = inp["W_attn"]
    b_attn = inp["b_attn"]
    W_attn2 = inp["W_attn2"]
    W_comb = inp["W_comb"]
    b_comb = inp["b_comb"]
    # b_attn2 shifts every score equally -> softmax-invariant, unused.

    wat = np.ascontiguousarray(W_attn.T)                    # [F, F] fp32
    # W1 part, h-chunked to [128, KH*F] bf16
    wat1 = np.ascontiguousarray(
        wat[:H].reshape(KH, 128, F).transpose(1, 0, 2)
        .reshape(128, KH * F)).astype(bf)
    # W2 part, scaled, laid out [128, (ft, kc, f)] fp8
    w2t8 = np.ascontiguousarray(
        (wat[H:] * W2SCALE).reshape(KH, 128, KF, 128)
        .transpose(1, 2, 0, 3).reshape(128, KF * KH * 128)).astype(f8)
    # W_attn2 scaled, [128, (ft, c)]: value = 256*W_attn2[ft*128+p] for all c
    w2r = (W2SCALE * W_attn2.reshape(KF, 128).T)            # [128, KF]
    w2rep8 = np.ascontiguousarray(
        np.broadcast_to(w2r[:, :, None], (128, KF, 128))
        .reshape(128, KF * 128)).astype(f8)
    wct = np.ascontiguousarray(W_comb.T).astype(bf)
    b_attn_2d = np.ascontiguousarray(b_attn.reshape(1, F))
    b_comb_2d = np.ascontiguousarray(b_comb.reshape(1, H))

    in_maps = []
    for c in range(NCORES):
        sl = slice(c * BL, (c + 1) * BL)
        # [S, BL, H] -> [BL, H, S] -> [BL, KH, 128, S] -> [128, BL, KH, S]
        enc = np.ascontiguousarray(
            encoder_states[:, sl, :].transpose(1, 2, 0)
            .reshape(BL, KH, 128, S).transpose(2, 0, 1, 3)
            .reshape(128, BL * KH * S))
        in_maps.append({
            "enc8": enc.astype(f8),
            "encb": enc.astype(bf),
            "w2t8": w2t8,
            "wat1": wat1,
            "w2rep8": w2rep8,
            "wct": wct,
            "hidT": np.ascontiguousarray(hidden[sl].T),
            "decT": np.ascontiguousarray(decoder_out[sl].T),
            "b_attn": b_attn_2d,
            "b_comb": b_comb_2d,
        })
    return in_maps


def kernel(**inputs):
    from concourse.bass_utils import run_bass_kernel_spmd

    in_maps = make_in_maps(inputs)
    nc = _get_nc()
    res = run_bass_kernel_spmd(nc, in_maps, list(range(NCORES)))
    out = np.concatenate([res.results[c]["out"] for c in range(NCORES)], axis=0)
    applied = np.concatenate(
        [np.ascontiguousarray(res.results[c]["appliedT"].T)
         for c in range(NCORES)], axis=0)
    return out.astype(np.float32), applied.astype(np.float32)


# revision 7
# speedup vs baseline: 1.8136x; 1.1517x over previous
"""Trainium2 Bass kernel for nn_AttentionModule (Bahdanau-style attention).

Reference computation (S=512, B=64, H=1024, F=2H):
    cat    = concat([hidden bcast to (S,B,H), encoder_states], -1)      [S,B,2H]
    scores = tanh(cat @ W_attn.T + b_attn) @ W_attn2.T + b_attn2        [S,B,1]
    attn   = softmax(scores[..., 0].T, axis=-1)                         [B,S]
    applied= einsum("bs,sbh->bh", attn, encoder_states)                 [B,H]
    out    = tanh(concat([decoder_out, applied], -1) @ W_comb.T + b_comb)

Sharding: data-parallel over B across 8 cores (8 batch rows per core).

v4 structure:
  - Main matmul T^T[f,s] = sum_h W2T[h,f]*encT[h,s] in fp8e4m3 DoubleRow
    (256 contraction rows per instruction).  W2 host-scaled by 256; the
    tanh undoes it via its fused scale=1/256.  W2T is laid out per-f-tile
    so the first DR group only waits on a 128KB DMA.
  - The first four DR groups of batch row 0 are emitted BEFORE the
    preamble so the PE has work while W1 streams in.
  - hid@W1.T preamble: W1 resident [128, KH*F], 16x256KB DMAs issued on
    the (idle) scalar queue in consumption order; 32 bf16 matmuls + PE
    transposes.  Scores bias hidb folded into tanh as per-partition bias.
  - Scores matmul (attn2) in bf16 with W_attn2 replicated to 128 columns:
    the psum scores tile [128, S] carries the row broadcast for free
    (fp8 tanh outputs measurably hurt the attention weights, so attn2
    stays bf16).
  - Softmax skips max-subtraction (scores are provably tiny); Exp fuses
    the sum via accum_out.
  - applied^T: bf16 mult+reduce per h-chunk on VectorE over a bf16
    encoder copy (gpsimd is too slow per-op to help).
  - enc fp8/bf16 copies streamed per batch row (prefetch one ahead).
  - Final combine bf16; decoder half emitted before the last batch row
    so the PE tail only waits on the last row's applied.
"""

import numpy as np

S, B, H = 512, 64, 1024
F = 2 * H
NCORES = 8
BL = B // NCORES          # 8 batch rows per core
KH = H // 128             # 8 contraction chunks over H
KF = F // 128             # 16 feature tiles
W2SCALE = 256.0           # host pre-scale on W2 for fp8 range

_CACHE = {}


def _build(num_devices=NCORES):
    from contextlib import ExitStack

    import concourse.tile as tile
    from concourse import bacc, mybir
    from concourse.masks import make_identity

    f32 = mybir.dt.float32
    bf16 = mybir.dt.bfloat16
    f8 = mybir.dt.float8e4
    AF = mybir.ActivationFunctionType
    ALU = mybir.AluOpType
    AX = mybir.AxisListType
    DR = mybir.MatmulPerfMode.DoubleRow

    nc = bacc.Bacc("TRN2", target_bir_lowering=False, debug=False,
                   num_devices=num_devices)

    # enc free layout per partition: [b, kc, s]; h = kc*128 + p
    enc8_d = nc.dram_tensor("enc8", [128, BL * KH * S], f8,
                            kind="ExternalInput").ap()
    encb_d = nc.dram_tensor("encb", [128, BL * KH * S], bf16,
                            kind="ExternalInput").ap()
    # w2t8 free layout: [ft, kc, f]; wat1 free layout: [kc, f]
    w2t8_d = nc.dram_tensor("w2t8", [128, KF * KH * 128], f8,
                            kind="ExternalInput").ap()
    wat1_d = nc.dram_tensor("wat1", [128, KH * F], bf16,
                            kind="ExternalInput").ap()
    wct = nc.dram_tensor("wct", [F, H], bf16, kind="ExternalInput").ap()
    hidT = nc.dram_tensor("hidT", [H, BL], f32, kind="ExternalInput").ap()
    decT = nc.dram_tensor("decT", [H, BL], f32, kind="ExternalInput").ap()
    w2rep = nc.dram_tensor("w2rep", [F, 128], f32, kind="ExternalInput").ap()
    b_attn_d = nc.dram_tensor("b_attn", [1, F], f32, kind="ExternalInput").ap()
    b_comb_d = nc.dram_tensor("b_comb", [1, H], f32, kind="ExternalInput").ap()
    out_d = nc.dram_tensor("out", [BL, H], f32, kind="ExternalOutput").ap()
    appT_d = nc.dram_tensor("appliedT", [H, BL], f32,
                            kind="ExternalOutput").ap()

    with tile.TileContext(nc) as tc:
        with ExitStack() as ctx:
            consts = ctx.enter_context(tc.tile_pool(name="consts", bufs=1))
            enc8_pool = ctx.enter_context(tc.tile_pool(name="enc8p", bufs=2))
            encb_pool = ctx.enter_context(tc.tile_pool(name="encbp", bufs=2))
            tanh_pool = ctx.enter_context(tc.tile_pool(name="tanh", bufs=18))
            attn_pool = ctx.enter_context(tc.tile_pool(name="attn", bufs=2))
            scr_pool = ctx.enter_context(tc.tile_pool(name="scr", bufs=2))
            small_pool = ctx.enter_context(tc.tile_pool(name="small", bufs=4))
            wct_pool = ctx.enter_context(tc.tile_pool(name="wct", bufs=4))
            psT_pool = ctx.enter_context(
                tc.tile_pool(name="psT", bufs=4, space="PSUM"))
            psSc_pool = ctx.enter_context(
                tc.tile_pool(name="psSc", bufs=2, space="PSUM"))
            psPre_pool = ctx.enter_context(
                tc.tile_pool(name="psPre", bufs=2, space="PSUM"))

            # ---- W1 on the scalar queue (idle early), consumption order ----
            w1_sb = consts.tile([128, KH * F], bf16)
            for j in range(2):
                for kc in range(KH):
                    nc.scalar.dma_start(
                        w1_sb[:, kc * F + j * 1024: kc * F + (j + 1) * 1024],
                        wat1_d[:, kc * F + j * 1024: kc * F + (j + 1) * 1024])

            # ---- sync queue: W2 head chunk, enc b0, then the rest ----
            w2t8_sb = consts.tile([128, KF * KH * 128], f8)
            CW = KH * 128

            def w2_load(ft):
                nc.sync.dma_start(w2t8_sb[:, ft * CW:(ft + 1) * CW],
                                  w2t8_d[:, ft * CW:(ft + 1) * CW])

            def enc8_load(b):
                t = enc8_pool.tile([128, KH * S], f8, tag="enc8", name="enc8")
                half = KH * S // 2
                for i in range(2):
                    nc.sync.dma_start(
                        t[:, i * half:(i + 1) * half],
                        enc8_d[:, b * KH * S + i * half:
                               b * KH * S + (i + 1) * half])
                return t

            def encb_load(b):
                t = encb_pool.tile([128, KH * S], bf16, tag="encb",
                                   name="encb")
                q = KH * S // 4
                for i in range(4):
                    nc.sync.dma_start(
                        t[:, i * q:(i + 1) * q],
                        encb_d[:, b * KH * S + i * q:
                               b * KH * S + (i + 1) * q])
                return t

            w2_load(0)
            enc8_tiles = {0: enc8_load(0)}
            for ft in range(1, 4):
                w2_load(ft)

            # ---- small constants (shipped fp32, cast on device) ----
            identity = consts.tile([128, 128], f32)
            make_identity(nc, identity[:])
            ones_bf = consts.tile([1, BL], bf16)
            nc.vector.memset(ones_bf[:], 1.0)
            hidT_32 = consts.tile([128, KH * BL], f32)
            for kc in range(KH):
                nc.sync.dma_start(hidT_32[:, kc * BL:(kc + 1) * BL],
                                  hidT[kc * 128:(kc + 1) * 128, :])
            hidT_sb = consts.tile([128, KH * BL], bf16)
            nc.vector.tensor_copy(hidT_sb[:], hidT_32[:])
            b_attn_32 = consts.tile([1, F], f32)
            nc.sync.dma_start(b_attn_32[:], b_attn_d[:])
            b_attn_sb = consts.tile([1, F], bf16)
            nc.vector.tensor_copy(b_attn_sb[:], b_attn_32[:])
            w2rep_32 = consts.tile([128, KF * 128], f32)
            for i in range(4):
                nc.sync.dma_start(
                    w2rep_32[:, i * 512:(i + 1) * 512].rearrange(
                        "p (a c) -> p a c", a=4),
                    w2rep[i * 512:(i + 1) * 512, :].rearrange(
                        "(a p) c -> p a c", p=128))
            w2rep_sb = consts.tile([128, KF * 128], bf16)
            nc.vector.tensor_copy(w2rep_sb[:], w2rep_32[:])

            for ft in range(4, KF):
                w2_load(ft)

            decT_32 = consts.tile([128, KH * BL], f32)
            for kc in range(KH):
                nc.sync.dma_start(decT_32[:, kc * BL:(kc + 1) * BL],
                                  decT[kc * 128:(kc + 1) * 128, :])
            decT_sb = consts.tile([128, KH * BL], bf16)
            nc.vector.tensor_copy(decT_sb[:], decT_32[:])
            b_comb_32 = consts.tile([1, H], f32)
            nc.sync.dma_start(b_comb_32[:], b_comb_d[:])
            b_comb_sb = consts.tile([1, H], bf16)
            nc.vector.tensor_copy(b_comb_sb[:], b_comb_32[:])
            encb_tiles = {0: encb_load(0)}

            hidbT_sb = consts.tile([128, KF * BL], f32)
            appT_sb = consts.tile([128, KH * BL], f32)
            appT_bf = consts.tile([128, KH * BL], bf16)
            w2t8_v = w2t8_sb.rearrange("p (t k f) -> p t k f", t=KF, k=KH)

            # ---- prologue: first 4 DR groups of b0 keep the PE busy while
            # W1 streams in for the preamble ----
            def dr_group(et8_v, ft):
                pT = psT_pool.tile([128, S], f32, tag="pT", name="pT")
                for kc2 in range(KH // 2):
                    nc.tensor.matmul(
                        pT[:],
                        w2t8_v[:, ft, 2 * kc2:2 * kc2 + 2, :],
                        et8_v[:, 2 * kc2:2 * kc2 + 2, :],
                        start=(kc2 == 0), stop=(kc2 == KH // 2 - 1),
                        perf_mode=DR)
                return pT

            et8_b0 = enc8_tiles.pop(0)
            et8_b0_v = et8_b0.rearrange("p (k s) -> p k s", k=KH)
            pT_pending = [dr_group(et8_b0_v, ft) for ft in range(4)]

            # ---- hid_part preamble: hidb[b, f] = hidden @ W1.T + b_attn ----
            hidb_row = consts.tile([BL, F], f32)
            for fc in range(F // 512):
                ph = psPre_pool.tile([BL, 512], f32, tag="pre", name=f"ph{fc}")
                for kc in range(KH):
                    nc.tensor.matmul(
                        ph[:], hidT_sb[:, kc * BL:(kc + 1) * BL],
                        w1_sb[:, kc * F + fc * 512: kc * F + (fc + 1) * 512],
                        start=(kc == 0), stop=False)
                nc.tensor.matmul(
                    ph[:], ones_bf[:], b_attn_sb[:, fc * 512:(fc + 1) * 512],
                    start=False, stop=True)
                nc.vector.tensor_copy(hidb_row[:, fc * 512:(fc + 1) * 512],
                                      ph[:])
            for ft in range(KF):
                ptp = psPre_pool.tile([128, BL], f32, tag="pre", name="ptp")
                nc.tensor.transpose(ptp[:],
                                    hidb_row[:, ft * 128:(ft + 1) * 128],
                                    identity[:BL, :BL])
                nc.vector.tensor_copy(hidbT_sb[:, ft * BL:(ft + 1) * BL],
                                      ptp[:])

            # ---- main loop over local batch rows ----
            inv = 1.0 / W2SCALE
            pouts = [None, None]
            for b in range(BL):
                if b + 1 < BL:
                    enc8_tiles[b + 1] = enc8_load(b + 1)
                    encb_tiles[b + 1] = encb_load(b + 1)
                if b > 0:
                    et8 = enc8_tiles.pop(b)
                    et8_v = et8.rearrange("p (k s) -> p k s", k=KH)
                else:
                    et8_v = et8_b0_v

                psc = psSc_pool.tile([128, S], f32, tag="psc", name="psc")
                th = [None] * KF

                def attn2(ft):
                    nc.tensor.matmul(
                        psc[:],
                        w2rep_sb[:, ft * 128:(ft + 1) * 128],
                        th[ft][:],
                        start=(ft == 0), stop=(ft == KF - 1))

                for ft in range(KF):
                    if b == 0 and ft < 4:
                        pT = pT_pending[ft]
                    else:
                        pT = dr_group(et8_v, ft)
                    t = tanh_pool.tile([128, S], bf16, tag="tanh", name="tanh")
                    nc.scalar.activation(
                        t[:], pT[:], AF.Tanh,
                        bias=hidbT_sb[:, ft * BL + b: ft * BL + b + 1],
                        scale=inv)
                    th[ft] = t
                    if ft >= 2:
                        attn2(ft - 2)
                attn2(KF - 2)
                attn2(KF - 1)

                # softmax over s (no max subtraction: |scores| <~ 2)
                attn = attn_pool.tile([128, S], bf16, tag="attn", name="attn")
                sumexp = small_pool.tile([128, 1], f32, tag="sumexp",
                                         name="sumexp")
                nc.scalar.activation(attn[:], psc[:], AF.Exp,
                                     bias=0.0, scale=1.0,
                                     accum_out=sumexp[:])
                recip = small_pool.tile([128, 1], f32, tag="recip",
                                        name="recip")
                nc.vector.reciprocal(recip[:], sumexp[:])
                nc.vector.tensor_scalar_mul(attn[:], attn[:], recip[:])

                # applied^T[h, b]: bf16 mult+reduce on VectorE
                etb = encb_tiles.pop(b)
                etb_v = etb.rearrange("p (k s) -> p k s", k=KH)
                for kc in range(KH):
                    scr = scr_pool.tile([128, S], bf16, tag="scr", name="scr")
                    nc.vector.tensor_tensor(out=scr[:], in0=etb_v[:, kc, :],
                                            in1=attn[:], op=ALU.mult)
                    nc.vector.reduce_sum(
                        appT_sb[:, kc * BL + b: kc * BL + b + 1],
                        scr[:], axis=AX.X)

                # decoder half of the final combine before the last row
                if b == BL - 2:
                    for i in range(2):
                        pouts[i] = psPre_pool.tile([BL, 512], f32, tag="pre",
                                                   name=f"po{i}")
                    for kc in range(KH):
                        w = wct_pool.tile([128, H], bf16, tag="wct",
                                          name="wctt")
                        nc.sync.dma_start(w[:], wct[kc * 128:(kc + 1) * 128, :])
                        for fc in range(2):
                            nc.tensor.matmul(
                                pouts[fc][:],
                                decT_sb[:, kc * BL:(kc + 1) * BL],
                                w[:, fc * 512:(fc + 1) * 512],
                                start=(kc == 0), stop=False)

            # ---- final combine: += applied @ Wc_applied.T, bias, tanh ----
            nc.vector.tensor_copy(appT_bf[:], appT_sb[:])
            for kc in range(KH):
                w = wct_pool.tile([128, H], bf16, tag="wct", name="wctt")
                nc.sync.dma_start(w[:], wct[(KH + kc) * 128:
                                            (KH + kc + 1) * 128, :])
                for fc in range(2):
                    nc.tensor.matmul(
                        pouts[fc][:], appT_bf[:, kc * BL:(kc + 1) * BL],
                        w[:, fc * 512:(fc + 1) * 512],
                        start=False, stop=False)
            for fc in range(2):
                nc.tensor.matmul(
                    pouts[fc][:], ones_bf[:],
                    b_comb_sb[:, fc * 512:(fc + 1) * 512],
                    start=False, stop=True)

            out_sb = consts.tile([BL, H], f32)
            for fc in range(2):
                nc.scalar.activation(out_sb[:, fc * 512:(fc + 1) * 512],
                                     pouts[fc][:], AF.Tanh)
            nc.sync.dma_start(out_d[:], out_sb[:])
            for kc in range(KH):
                nc.sync.dma_start(appT_d[kc * 128:(kc + 1) * 128, :],
                                  appT_sb[:, kc * BL:(kc + 1) * BL])

    nc.compile()
    return nc


def _get_nc():
    if "nc" not in _CACHE:
        _CACHE["nc"] = _build()
    return _CACHE["nc"]


def make_in_maps(inputs):
    import ml_dtypes
    bf = ml_dtypes.bfloat16
    f8 = ml_dtypes.float8_e4m3

    inp = {k: np.asarray(v, dtype=np.float32) for k, v in inputs.items()}
    hidden = inp["hidden"]
    decoder_out = inp["decoder_out"]
    encoder_states = inp["encoder_states"]
    W_attn = inp["W_attn"]
    b_attn = inp["b_attn"]
    W_attn2 = inp["W_attn2"]
    W_comb = inp["W_comb"]
    b_comb = inp["b_comb"]
    # b_attn2 shifts every score equally -> softmax-invariant, unused.

    wat = np.ascontiguousarray(W_attn.T)                    # [F, F] fp32
    wat1 = np.ascontiguousarray(
        wat[:H].reshape(KH, 128, F).transpose(1, 0, 2)
        .reshape(128, KH * F)).astype(bf)
    w2t8 = np.ascontiguousarray(
        (wat[H:] * W2SCALE).reshape(KH, 128, KF, 128)
        .transpose(1, 2, 0, 3).reshape(128, KF * KH * 128)).astype(f8)
    wct = np.ascontiguousarray(W_comb.T).astype(bf)
    w2rep = np.ascontiguousarray(np.repeat(W_attn2.reshape(F, 1), 128, axis=1))
    b_attn_2d = np.ascontiguousarray(b_attn.reshape(1, F))
    b_comb_2d = np.ascontiguousarray(b_comb.reshape(1, H))

    in_maps = []
    for c in range(NCORES):
        sl = slice(c * BL, (c + 1) * BL)
        # [S, BL, H] -> [BL, H, S] -> [BL, KH, 128, S] -> [128, BL, KH, S]
        enc = np.ascontiguousarray(
            encoder_states[:, sl, :].transpose(1, 2, 0)
            .reshape(BL, KH, 128, S).transpose(2, 0, 1, 3)
            .reshape(128, BL * KH * S))
        in_maps.append({
            "enc8": enc.astype(f8),
            "encb": enc.astype(bf),
            "w2t8": w2t8,
            "wat1": wat1,
            "wct": wct,
            "hidT": np.ascontiguousarray(hidden[sl].T),
            "decT": np.ascontiguousarray(decoder_out[sl].T),
            "w2rep": w2rep,
            "b_attn": b_attn_2d,
            "b_comb": b_comb_2d,
        })
    return in_maps


def kernel(**inputs):
    from concourse.bass_utils import run_bass_kernel_spmd

    in_maps = make_in_maps(inputs)
    nc = _get_nc()
    res = run_bass_kernel_spmd(nc, in_maps, list(range(NCORES)))
    out = np.concatenate([res.results[c]["out"] for c in range(NCORES)], axis=0)
    applied = np.concatenate(
        [np.ascontiguousarray(res.results[c]["appliedT"].T)
         for c in range(NCORES)], axis=0)
    return out.astype(np.float32), applied.astype(np.float32)


# revision 17
# speedup vs baseline: 2.0009x; 1.1032x over previous
"""Trainium2 Bass kernel for nn_AttentionModule (Bahdanau-style attention).

Reference computation (S=512, B=64, H=1024, F=2H):
    cat    = concat([hidden bcast to (S,B,H), encoder_states], -1)      [S,B,2H]
    scores = tanh(cat @ W_attn.T + b_attn) @ W_attn2.T + b_attn2        [S,B,1]
    attn   = softmax(scores[..., 0].T, axis=-1)                         [B,S]
    applied= einsum("bs,sbh->bh", attn, encoder_states)                 [B,H]
    out    = tanh(concat([decoder_out, applied], -1) @ W_comb.T + b_comb)

Sharding: data-parallel over B across 8 cores (8 batch rows per core).

v4 structure:
  - Main matmul T^T[f,s] = sum_h W2T[h,f]*encT[h,s] in fp8e4m3 DoubleRow
    (256 contraction rows per instruction).  W2 host-scaled by 256; the
    tanh undoes it via its fused scale=1/256.  W2T is laid out per-f-tile
    so the first DR group only waits on a 128KB DMA.
  - The first four DR groups of batch row 0 are emitted BEFORE the
    preamble so the PE has work while W1 streams in.
  - hid@W1.T preamble: W1 resident [128, KH*F], 16x256KB DMAs issued on
    the (idle) scalar queue in consumption order; 32 bf16 matmuls + PE
    transposes.  Scores bias hidb folded into tanh as per-partition bias.
  - Scores matmul (attn2) in bf16 with W_attn2 replicated to 128 columns:
    the psum scores tile [128, S] carries the row broadcast for free
    (fp8 tanh outputs measurably hurt the attention weights, so attn2
    stays bf16).
  - Softmax skips max-subtraction (scores are provably tiny); Exp fuses
    the sum via accum_out.
  - applied^T: bf16 mult+reduce per h-chunk on VectorE over a bf16
    encoder copy (gpsimd is too slow per-op to help).
  - enc fp8/bf16 copies streamed per batch row (prefetch one ahead).
  - Final combine bf16; decoder half emitted before the last batch row
    so the PE tail only waits on the last row's applied.
"""

import numpy as np

S, B, H = 512, 64, 1024
F = 2 * H
NCORES = 8
BL = B // NCORES          # 8 batch rows per core
KH = H // 128             # 8 contraction chunks over H
KF = F // 128             # 16 feature tiles
W2SCALE = 256.0           # host pre-scale on W2 for fp8 range

_CACHE = {}


def _build(num_devices=NCORES):
    from contextlib import ExitStack

    import concourse.tile as tile
    from concourse import bacc, mybir
    from concourse.masks import make_identity

    f32 = mybir.dt.float32
    bf16 = mybir.dt.bfloat16
    f8 = mybir.dt.float8e4
    AF = mybir.ActivationFunctionType
    ALU = mybir.AluOpType
    AX = mybir.AxisListType
    DR = mybir.MatmulPerfMode.DoubleRow

    nc = bacc.Bacc("TRN2", target_bir_lowering=False, debug=False,
                   num_devices=num_devices)

    # enc free layout per partition: [b, kc, s]; h = kc*128 + p
    enc8_d = nc.dram_tensor("enc8", [128, BL * KH * S], f8,
                            kind="ExternalInput").ap()
    encb_d = nc.dram_tensor("encb", [128, BL * KH * S], bf16,
                            kind="ExternalInput").ap()
    # w2t8 free layout: [ft, kc, f]; wat1 free layout: [kc, f]
    w2t8_d = nc.dram_tensor("w2t8", [128, KF * KH * 128], f8,
                            kind="ExternalInput").ap()
    wat1_d = nc.dram_tensor("wat1", [128, KH * F], bf16,
                            kind="ExternalInput").ap()
    wct = nc.dram_tensor("wct", [F, H], bf16, kind="ExternalInput").ap()
    hidT = nc.dram_tensor("hidT", [H, BL], f32, kind="ExternalInput").ap()
    decT = nc.dram_tensor("decT", [H, BL], f32, kind="ExternalInput").ap()
    w2rep = nc.dram_tensor("w2rep", [128, KF * 128], f8,
                           kind="ExternalInput").ap()
    b_attn_d = nc.dram_tensor("b_attn", [1, F], f32, kind="ExternalInput").ap()
    b_comb_d = nc.dram_tensor("b_comb", [1, H], f32, kind="ExternalInput").ap()
    out_d = nc.dram_tensor("out", [BL, H], f32, kind="ExternalOutput").ap()
    appT_d = nc.dram_tensor("appliedT", [H, BL], f32,
                            kind="ExternalOutput").ap()

    with tile.TileContext(nc) as tc:
        with ExitStack() as ctx:
            consts = ctx.enter_context(tc.tile_pool(name="consts", bufs=1))
            enc8_pool = ctx.enter_context(tc.tile_pool(name="enc8p", bufs=2))
            encb_pool = ctx.enter_context(tc.tile_pool(name="encbp", bufs=2))
            tanh_pool = ctx.enter_context(tc.tile_pool(name="tanh", bufs=18))
            attn_pool = ctx.enter_context(tc.tile_pool(name="attn", bufs=2))
            scr_pool = ctx.enter_context(tc.tile_pool(name="scr", bufs=2))
            small_pool = ctx.enter_context(tc.tile_pool(name="small", bufs=4))
            wct_pool = ctx.enter_context(tc.tile_pool(name="wct", bufs=4))
            wcta_pool = ctx.enter_context(tc.tile_pool(name="wcta", bufs=8))
            psT_pool = ctx.enter_context(
                tc.tile_pool(name="psT", bufs=4, space="PSUM"))
            psSc_pool = ctx.enter_context(
                tc.tile_pool(name="psSc", bufs=2, space="PSUM"))
            psPre_pool = ctx.enter_context(
                tc.tile_pool(name="psPre", bufs=2, space="PSUM"))

            # ---- W1 on the scalar queue (idle early), consumption order ----
            w1_sb = consts.tile([128, KH * F], bf16)
            for j in range(2):
                for kc in range(KH):
                    nc.scalar.dma_start(
                        w1_sb[:, kc * F + j * 1024: kc * F + (j + 1) * 1024],
                        wat1_d[:, kc * F + j * 1024: kc * F + (j + 1) * 1024])

            # ---- sync queue: W2 head chunk, enc b0, then the rest ----
            w2t8_sb = consts.tile([128, KF * KH * 128], f8)
            CW = KH * 128

            def w2_load(ft):
                nc.sync.dma_start(w2t8_sb[:, ft * CW:(ft + 1) * CW],
                                  w2t8_d[:, ft * CW:(ft + 1) * CW])

            def enc8_load(b):
                t = enc8_pool.tile([128, KH * S], f8, tag="enc8", name="enc8")
                half = KH * S // 2
                for i in range(2):
                    nc.sync.dma_start(
                        t[:, i * half:(i + 1) * half],
                        enc8_d[:, b * KH * S + i * half:
                               b * KH * S + (i + 1) * half])
                return t

            def encb_load(b):
                t = encb_pool.tile([128, KH * S], bf16, tag="encb",
                                   name="encb")
                q = KH * S // 4
                for i in range(4):
                    nc.sync.dma_start(
                        t[:, i * q:(i + 1) * q],
                        encb_d[:, b * KH * S + i * q:
                               b * KH * S + (i + 1) * q])
                return t

            w2_load(0)
            enc8_tiles = {0: enc8_load(0)}
            for ft in range(1, 4):
                w2_load(ft)

            # ---- small constants (shipped fp32, cast on device) ----
            identity = consts.tile([128, 128], f32)
            make_identity(nc, identity[:])
            ones_bf = consts.tile([1, BL], bf16)
            nc.vector.memset(ones_bf[:], 1.0)
            hidT_32 = consts.tile([128, KH * BL], f32)
            for kc in range(KH):
                nc.sync.dma_start(hidT_32[:, kc * BL:(kc + 1) * BL],
                                  hidT[kc * 128:(kc + 1) * 128, :])
            hidT_sb = consts.tile([128, KH * BL], bf16)
            nc.vector.tensor_copy(hidT_sb[:], hidT_32[:])
            b_attn_32 = consts.tile([1, F], f32)
            nc.sync.dma_start(b_attn_32[:], b_attn_d[:])
            b_attn_sb = consts.tile([1, F], bf16)
            nc.vector.tensor_copy(b_attn_sb[:], b_attn_32[:])
            w2rep8_sb = consts.tile([128, KF * 128], f8)
            nc.sync.dma_start(w2rep8_sb[:], w2rep[:])
            w2rep8_v = w2rep8_sb.rearrange("p (t c) -> p t c", t=KF)

            for ft in range(4, KF):
                w2_load(ft)

            decT_32 = consts.tile([128, KH * BL], f32)
            for kc in range(KH):
                nc.sync.dma_start(decT_32[:, kc * BL:(kc + 1) * BL],
                                  decT[kc * 128:(kc + 1) * 128, :])
            decT_sb = consts.tile([128, KH * BL], bf16)
            nc.vector.tensor_copy(decT_sb[:], decT_32[:])
            b_comb_32 = consts.tile([1, H], f32)
            nc.sync.dma_start(b_comb_32[:], b_comb_d[:])
            b_comb_sb = consts.tile([1, H], bf16)
            nc.vector.tensor_copy(b_comb_sb[:], b_comb_32[:])
            encb_tiles = {0: encb_load(0)}

            hidbT_sb = consts.tile([128, KF * BL], f32)
            appT_sb = consts.tile([128, KH * BL], f32)
            appT_bf = consts.tile([128, KH * BL], bf16)
            w2t8_v = w2t8_sb.rearrange("p (t k f) -> p t k f", t=KF, k=KH)

            # ---- prologue: first 4 DR groups of b0 keep the PE busy while
            # W1 streams in for the preamble ----
            def dr_group(et8_v, ft):
                pT = psT_pool.tile([128, S], f32, tag="pT", name="pT")
                for kc2 in range(KH // 2):
                    nc.tensor.matmul(
                        pT[:],
                        w2t8_v[:, ft, 2 * kc2:2 * kc2 + 2, :],
                        et8_v[:, 2 * kc2:2 * kc2 + 2, :],
                        start=(kc2 == 0), stop=(kc2 == KH // 2 - 1),
                        perf_mode=DR)
                return pT

            et8_b0 = enc8_tiles.pop(0)
            et8_b0_v = et8_b0.rearrange("p (k s) -> p k s", k=KH)
            pT_pending = [dr_group(et8_b0_v, ft) for ft in range(4)]

            # ---- hid_part preamble: hidb[b, f] = hidden @ W1.T + b_attn ----
            hidb_row = consts.tile([BL, F], f32)
            for fc in range(F // 512):
                ph = psPre_pool.tile([BL, 512], f32, tag="pre", name=f"ph{fc}")
                for kc in range(KH):
                    nc.tensor.matmul(
                        ph[:], hidT_sb[:, kc * BL:(kc + 1) * BL],
                        w1_sb[:, kc * F + fc * 512: kc * F + (fc + 1) * 512],
                        start=(kc == 0), stop=False)
                nc.tensor.matmul(
                    ph[:], ones_bf[:], b_attn_sb[:, fc * 512:(fc + 1) * 512],
                    start=False, stop=True)
                nc.vector.tensor_copy(hidb_row[:, fc * 512:(fc + 1) * 512],
                                      ph[:])
            for ft in range(KF):
                ptp = psPre_pool.tile([128, BL], f32, tag="pre", name="ptp")
                nc.tensor.transpose(ptp[:],
                                    hidb_row[:, ft * 128:(ft + 1) * 128],
                                    identity[:BL, :BL])
                nc.vector.tensor_copy(hidbT_sb[:, ft * BL:(ft + 1) * BL],
                                      ptp[:])

            # ---- main loop over local batch rows ----
            inv = 1.0 / W2SCALE
            pouts = [None, None]
            for b in range(BL):
                if b + 1 < BL:
                    enc8_tiles[b + 1] = enc8_load(b + 1)
                    encb_tiles[b + 1] = encb_load(b + 1)
                if b > 0:
                    et8 = enc8_tiles.pop(b)
                    et8_v = et8.rearrange("p (k s) -> p k s", k=KH)
                else:
                    et8_v = et8_b0_v

                psc = psSc_pool.tile([128, S], f32, tag="psc", name="psc")
                thp = [None] * (KF // 2)

                def attn2(fp):
                    nc.tensor.matmul(
                        psc[:],
                        w2rep8_v[:, 2 * fp:2 * fp + 2, :],
                        thp[fp].rearrange("p (t s) -> p t s", t=2),
                        start=(fp == 0), stop=(fp == KF // 2 - 1),
                        perf_mode=DR)

                for ft in range(KF):
                    if b == 0 and ft < 4:
                        pT = pT_pending[ft]
                    else:
                        pT = dr_group(et8_v, ft)
                    if ft % 2 == 0:
                        thp[ft // 2] = tanh_pool.tile([128, 2 * S], f8,
                                                      tag="tanh", name="tanh")
                    nc.scalar.activation(
                        thp[ft // 2][:, (ft % 2) * S:(ft % 2 + 1) * S],
                        pT[:], AF.Tanh,
                        bias=hidbT_sb[:, ft * BL + b: ft * BL + b + 1],
                        scale=inv)
                    if ft % 2 == 1 and ft >= 3:
                        attn2(ft // 2 - 1)
                attn2(KF // 2 - 2)
                attn2(KF // 2 - 1)

                # softmax over s (no max subtraction: |scores| <~ 2)
                attn = attn_pool.tile([128, S], bf16, tag="attn", name="attn")
                sumexp = small_pool.tile([128, 1], f32, tag="sumexp",
                                         name="sumexp")
                nc.scalar.activation(attn[:], psc[:], AF.Exp,
                                     bias=0.0, scale=inv,
                                     accum_out=sumexp[:])
                recip = small_pool.tile([128, 1], f32, tag="recip",
                                        name="recip")
                nc.vector.reciprocal(recip[:], sumexp[:])
                nc.vector.tensor_scalar_mul(attn[:], attn[:], recip[:])

                # applied^T[h, b]: bf16 mult+reduce on VectorE.  For the last
                # row, each finished h-chunk immediately feeds its final-
                # combine matmul so the PE tail never waits on the full row.
                etb = encb_tiles.pop(b)
                etb_v = etb.rearrange("p (k s) -> p k s", k=KH)
                for kc in range(KH):
                    scr = scr_pool.tile([128, S], bf16, tag="scr", name="scr")
                    nc.vector.tensor_tensor(out=scr[:], in0=etb_v[:, kc, :],
                                            in1=attn[:], op=ALU.mult)
                    nc.vector.reduce_sum(
                        appT_sb[:, kc * BL + b: kc * BL + b + 1],
                        scr[:], axis=AX.X)
                    if b == BL - 1:
                        nc.vector.tensor_copy(
                            appT_bf[:, kc * BL:(kc + 1) * BL],
                            appT_sb[:, kc * BL:(kc + 1) * BL])
                        w = wct_tiles[kc]
                        for fc in range(2):
                            nc.tensor.matmul(
                                pouts[fc][:],
                                appT_bf[:, kc * BL:(kc + 1) * BL],
                                w[:, fc * 512:(fc + 1) * 512],
                                start=False, stop=False)

                # decoder half of the final combine before the last row
                if b == BL - 2:
                    for i in range(2):
                        pouts[i] = psPre_pool.tile([BL, 512], f32, tag="pre",
                                                   name=f"po{i}")
                    for kc in range(KH):
                        w = wct_pool.tile([128, H], bf16, tag="wct",
                                          name="wctt")
                        nc.sync.dma_start(w[:], wct[kc * 128:(kc + 1) * 128, :])
                        for fc in range(2):
                            nc.tensor.matmul(
                                pouts[fc][:],
                                decT_sb[:, kc * BL:(kc + 1) * BL],
                                w[:, fc * 512:(fc + 1) * 512],
                                start=(kc == 0), stop=False)
                    # applied-half weights, loaded ahead of the last row
                    wct_tiles = []
                    for kc in range(KH):
                        w = wcta_pool.tile([128, H], bf16, tag="wcta",
                                           name="wcta")
                        nc.sync.dma_start(w[:], wct[(KH + kc) * 128:
                                                    (KH + kc + 1) * 128, :])
                        wct_tiles.append(w)

            # ---- final combine: bias, tanh ----
            for fc in range(2):
                nc.tensor.matmul(
                    pouts[fc][:], ones_bf[:],
                    b_comb_sb[:, fc * 512:(fc + 1) * 512],
                    start=False, stop=True)

            out_sb = consts.tile([BL, H], f32)
            for fc in range(2):
                nc.scalar.activation(out_sb[:, fc * 512:(fc + 1) * 512],
                                     pouts[fc][:], AF.Tanh)
            nc.sync.dma_start(out_d[:], out_sb[:])
            for kc in range(KH):
                nc.sync.dma_start(appT_d[kc * 128:(kc + 1) * 128, :],
                                  appT_sb[:, kc * BL:(kc + 1) * BL])

    nc.compile()
    return nc


def _get_nc():
    if "nc" not in _CACHE:
        _CACHE["nc"] = _build()
    return _CACHE["nc"]


def make_in_maps(inputs):
    import ml_dtypes
    bf = ml_dtypes.bfloat16
    f8 = ml_dtypes.float8_e4m3

    inp = {k: np.asarray(v, dtype=np.float32) for k, v in inputs.items()}
    hidden = inp["hidden"]
    decoder_out = inp["decoder_out"]
    encoder_states = inp["encoder_states"]
    W_attn = inp["W_attn"]
    b_attn = inp["b_attn"]
    W_attn2 = inp["W_attn2"]
    W_comb = inp["W_comb"]
    b_comb = inp["b_comb"]
    # b_attn2 shifts every score equally -> softmax-invariant, unused.

    wat = np.ascontiguousarray(W_attn.T)                    # [F, F] fp32
    wat1 = np.ascontiguousarray(
        wat[:H].reshape(KH, 128, F).transpose(1, 0, 2)
        .reshape(128, KH * F)).astype(bf)
    w2t8 = np.ascontiguousarray(
        (wat[H:] * W2SCALE).reshape(KH, 128, KF, 128)
        .transpose(1, 2, 0, 3).reshape(128, KF * KH * 128)).astype(f8)
    wct = np.ascontiguousarray(W_comb.T).astype(bf)
    # [128, (ft c)] with value 256*W_attn2[ft*128+p] replicated over c
    w2r = (W2SCALE * W_attn2.reshape(KF, 128).T).astype(np.float32)
    w2rep = np.ascontiguousarray(
        np.broadcast_to(w2r[:, :, None], (128, KF, 128))
        .reshape(128, KF * 128)).astype(f8)
    b_attn_2d = np.ascontiguousarray(b_attn.reshape(1, F))
    b_comb_2d = np.ascontiguousarray(b_comb.reshape(1, H))

    in_maps = []
    for c in range(NCORES):
        sl = slice(c * BL, (c + 1) * BL)
        # [S, BL, H] -> [BL, H, S] -> [BL, KH, 128, S] -> [128, BL, KH, S]
        enc = np.ascontiguousarray(
            encoder_states[:, sl, :].transpose(1, 2, 0)
            .reshape(BL, KH, 128, S).transpose(2, 0, 1, 3)
            .reshape(128, BL * KH * S))
        in_maps.append({
            "enc8": enc.astype(f8),
            "encb": enc.astype(bf),
            "w2t8": w2t8,
            "wat1": wat1,
            "wct": wct,
            "hidT": np.ascontiguousarray(hidden[sl].T),
            "decT": np.ascontiguousarray(decoder_out[sl].T),
            "w2rep": w2rep,
            "b_attn": b_attn_2d,
            "b_comb": b_comb_2d,
        })
    return in_maps


def kernel(**inputs):
    from concourse.bass_utils import run_bass_kernel_spmd

    in_maps = make_in_maps(inputs)
    nc = _get_nc()
    res = run_bass_kernel_spmd(nc, in_maps, list(range(NCORES)))
    out = np.concatenate([res.results[c]["out"] for c in range(NCORES)], axis=0)
    applied = np.concatenate(
        [np.ascontiguousarray(res.results[c]["appliedT"].T)
         for c in range(NCORES)], axis=0)
    return out.astype(np.float32), applied.astype(np.float32)
